# revision 1
# baseline (speedup 1.0000x reference)
"""Optimized decoder kernel: char-embed -> conv+maxpool -> 6-layer BERT
decoder -> nar proj -> char LSTM -> NLL loss.

Single-pass full-batch implementation (B=32). All large GEMMs are shaped
as single 2-D sgemm calls ([B*T, D] @ [D, N]) so BLAS runs at full
efficiency; the LSTM recurrence runs once over 1000 steps at B=32 with
preallocated buffers and in-place transcendentals.
"""

import numpy as np

DIM = 1024
HEADS = 16
HD = DIM // HEADS
FF = 2048
E = 128
V = 256
SHRINK = 5
NLAYERS = 6
HALF = DIM // 2
NEG = np.float32(-1e9)


def _ln(x, g, b):
    m = x.mean(-1, keepdims=True, dtype=np.float32)
    x = x - m
    v = np.einsum('...d,...d->...', x, x, dtype=np.float32) / x.shape[-1]
    np.sqrt(v + np.float32(1e-12), out=v)
    x /= v[..., None]
    x *= g
    x += b
    return x


def _softmax_ip(x):
    # in-place softmax over last axis
    x -= x.max(-1, keepdims=True)
    np.exp(x, out=x)
    x /= x.sum(-1, keepdims=True, dtype=np.float32)
    return x


def _sigmoid_ip(x):
    # in-place sigmoid
    np.negative(x, out=x)
    np.exp(x, out=x)
    x += np.float32(1.0)
    np.reciprocal(x, out=x)
    return x


def kernel(**d):
    f32 = np.float32
    inputs = {}
    for k, v in d.items():
        v = np.asarray(v)
        if v.dtype == np.float64:
            v = v.astype(f32)
        inputs[k] = v
    d = inputs

    encoder_states = d["encoder_states"]; encoder_mask = d["encoder_mask"]
    target_ids = d["target_ids"]; target_mask = d["target_mask"]
    char_emb = d["char_emb"]; pre_pos_emb = d["pre_pos_emb"]
    Wc = d["Wc"]; bc = d["bc"]
    bert_pos_emb = d["bert_pos_emb"]; bert_tok_emb = d["bert_tok_emb"]
    emb_ln_g = d["emb_ln_g"]; emb_ln_b = d["emb_ln_b"]
    Wqkv = d["Wqkv"]; bqkv = d["bqkv"]; Wo = d["Wo"]; bo = d["bo"]
    ln1_g = d["ln1_g"]; ln1_b = d["ln1_b"]
    Cq = d["Cq"]; cbq = d["cbq"]; Ckv = d["Ckv"]; cbkv = d["cbkv"]
    Co = d["Co"]; cbo = d["cbo"]; ln2_g = d["ln2_g"]; ln2_b = d["ln2_b"]
    W1 = d["W1"]; b1 = d["b1"]; W2 = d["W2"]; b2 = d["b2"]
    ln3_g = d["ln3_g"]; ln3_b = d["ln3_b"]
    Wnar = d["Wnar"]; bnar = d["bnar"]
    W_ih = d["W_ih"]; W_hh = d["W_hh"]; b_ih = d["b_ih"]; b_hh = d["b_hh"]
    Wout = d["Wout"]; bout = d["bout"]

    B, S = target_ids.shape
    T = S // SHRINK
    SRC = encoder_states.shape[1]

    dec_input = np.concatenate(
        [np.ones((B, SHRINK), target_ids.dtype), target_ids[:, :-SHRINK]], 1)
    input_mask = np.concatenate(
        [np.ones((B, SHRINK), target_mask.dtype), target_mask[:, :-SHRINK]], 1)

    emb = char_emb[dec_input]
    emb += pre_pos_emb[0, :S]

    # position-wise proj + relu + mask, then max-pool over SHRINK window
    h = emb.reshape(B * S, E) @ Wc
    h += bc
    np.maximum(h, f32(0.0), out=h)
    h = h.reshape(B, S, DIM)
    h *= input_mask[..., None]
    pooled = h.reshape(B, T, SHRINK, DIM).max(2)
    smask = input_mask.reshape(B, T, SHRINK).max(2)

    x = pooled
    x += bert_pos_emb[:T]
    x += bert_tok_emb
    x = _ln(x, emb_ln_g, emb_ln_b)

    causal_bias = np.where(np.tril(np.ones((T, T), bool)), f32(0.0), NEG)
    self_bias = causal_bias[None] + (f32(1.0) - smask)[:, None, :] * NEG  # [B,T,T]
    cross_bias = ((f32(1.0) - encoder_mask)[:, None, :] * NEG)           # [B,1,SRC]
    scale = f32(1.0 / np.sqrt(HD))

    BT = B * T
    enc2 = np.ascontiguousarray(encoder_states.reshape(B * SRC, DIM))

    for l in range(NLAYERS):
        # ---- self attention ----
        qkv = x.reshape(BT, DIM) @ Wqkv[l]
        qkv += bqkv[l]
        qkv = qkv.reshape(B, T, 3, HEADS, HD).transpose(2, 0, 3, 1, 4)
        q, k, v = qkv[0], qkv[1], qkv[2]           # [B,H,T,HD]
        s = np.matmul(q, k.transpose(0, 1, 3, 2))  # [B,H,T,T]
        s *= scale
        s += self_bias[:, None]
        a = _softmax_ip(s)
        o = np.matmul(a, v)                        # [B,H,T,HD]
        o = np.ascontiguousarray(o.transpose(0, 2, 1, 3)).reshape(BT, DIM)
        o = o @ Wo[l]
        o += bo[l]
        x = _ln(x + o.reshape(B, T, DIM), ln1_g[l], ln1_b[l])

        # ---- cross attention ----
        q = x.reshape(BT, DIM) @ Cq[l]
        q += cbq[l]
        q = q.reshape(B, T, HEADS, HD).transpose(0, 2, 1, 3)
        kv = enc2 @ Ckv[l]
        kv += cbkv[l]
        kv = kv.reshape(B, SRC, 2, HEADS, HD).transpose(2, 0, 3, 1, 4)
        k, v = kv[0], kv[1]                        # [B,H,SRC,HD]
        s = np.matmul(q, k.transpose(0, 1, 3, 2))  # [B,H,T,SRC]
        s *= scale
        s += cross_bias[:, None]
        a = _softmax_ip(s)
        o = np.matmul(a, v)
        o = np.ascontiguousarray(o.transpose(0, 2, 1, 3)).reshape(BT, DIM)
        o = o @ Co[l]
        o += cbo[l]
        x = _ln(x + o.reshape(B, T, DIM), ln2_g[l], ln2_b[l])

        # ---- FFN ----
        f = x.reshape(BT, DIM) @ W1[l]
        f += b1[l]
        np.maximum(f, f32(0.0), out=f)
        f = f @ W2[l]
        f += b2[l]
        x = _ln(x + f.reshape(B, T, DIM), ln3_g[l], ln3_b[l])

    # nar proj -> per-char states [B,S,HALF]
    cs = x.reshape(BT, DIM) @ Wnar
    cs += bnar
    char_states = cs.reshape(B, T * SHRINK, HALF)[:, :S]

    # char LSTM over the full target sequence; build lstm_in time-major so
    # the input GEMM lands directly in [S,B,4D] (no 512MB x_part transpose)
    pad_in = np.concatenate(
        [np.ones((B, 1), target_ids.dtype), target_ids], 1)[:, :S]
    lstm_in_tm = np.empty((S, B, E + HALF), f32)
    lstm_in_tm[:, :, :E] = char_emb[pad_in.T]
    lstm_in_tm[:, :, E:] = char_states.transpose(1, 0, 2)

    x_part = lstm_in_tm.reshape(S * B, E + HALF) @ W_ih.T
    x_part += b_ih + b_hh
    x_part = x_part.reshape(S, B, 4 * DIM)

    W_hh_T = np.ascontiguousarray(W_hh.T)          # [DIM, 4*DIM]
    hs = np.empty((S, B, DIM), f32)
    hprev = np.zeros((B, DIM), f32)
    c = np.zeros((B, DIM), f32)
    g = np.empty((B, 4 * DIM), f32)
    tmp = np.empty((B, DIM), f32)
    for t in range(S):
        np.dot(hprev, W_hh_T, out=g)
        g += x_part[t]
        i = _sigmoid_ip(g[:, :DIM])
        fgate = _sigmoid_ip(g[:, DIM:2 * DIM])
        gg = np.tanh(g[:, 2 * DIM:3 * DIM], out=g[:, 2 * DIM:3 * DIM])
        o = _sigmoid_ip(g[:, 3 * DIM:])
        c *= fgate
        np.multiply(i, gg, out=tmp)
        c += tmp
        hprev = np.tanh(c, out=hs[t])
        hprev *= o
    # logits stay time-major: hs is [S,B,D] contiguous and the time-major
    # char_states slab already exists inside lstm_in_tm
    feats = np.concatenate([hs, lstm_in_tm[:, :, E:]], -1)  # [S,B,DIM+HALF]
    logits = feats.reshape(S * B, DIM + HALF) @ Wout
    logits += bout

    # nll = logsumexp(logits) - logits[target]
    m = logits.max(-1, keepdims=True)
    logits -= m
    picked = np.take_along_axis(
        logits, target_ids.T.reshape(S * B, 1).astype(np.int64), 1)[:, 0].copy()
    np.exp(logits, out=logits)
    lse = np.log(logits.sum(-1, dtype=np.float32))
    nll = (lse - picked).reshape(S, B)

    msum = target_mask.sum(dtype=np.float64)
    return np.float32((nll * target_mask.T).sum(dtype=np.float64) / msum)



# revision 2
# speedup vs baseline: 2.3057x; 2.3057x over previous
"""Trainium2 Bass kernel for the char-decoder model (8 NeuronCores, SPMD).

Device: conv front-end + 6-layer BERT decoder + nar projection, data-parallel
over batch (4/core), feature-major activations, bf16 GEMMs with fp32 PSUM.
Host: char-LSTM recurrence + output projection/NLL (numpy), pending the
tensor-parallel on-device LSTM.
"""

import os
import numpy as np
import ml_dtypes

import concourse.bass as bass
import concourse.bacc as bacc
import concourse.mybir as mybir
import concourse.tile as tile
from concourse.bass import ds, ts
from concourse.bass_utils import run_bass_kernel_spmd
from concourse.masks import make_identity

F32 = mybir.dt.float32
BF16 = mybir.dt.bfloat16
AF = mybir.ActivationFunctionType
ALU = mybir.AluOpType
BF = ml_dtypes.bfloat16

DIM, HEADS, HD, FF, E, V, SHRINK, L, HALF = 1024, 16, 64, 2048, 128, 256, 5, 6, 512
B, S, T, SRC, NCORE = 32, 1000, 200, 256, 8
BC = B // NCORE            # 4 batches/core
TOK = BC * T               # 800
ETOK = BC * SRC            # 1024
CTOK = BC * S              # 4000
SCALE = 1.0 / 8.0

_CACHE = {}


def _tokwins(n, w=512):
    o = 0
    while o < n:
        yield o, min(w, n - o)
        o += w


def build_program():
    nc = bacc.Bacc(None, target_bir_lowering=False, num_devices=NCORE)

    def din(name, shape, dt=BF16):
        return nc.dram_tensor(name, shape, dt, kind="ExternalInput")

    embT = din("embT", [E, CTOK])
    convmask = din("convmask", [1, CTOK], F32)
    posT = din("posT", [128, 8, TOK])
    selfmaskT = din("selfmaskT", [128, 2, BC, T])
    crossmaskT = din("crossmaskT", [128, 2, BC], F32)
    encT = din("encT", [128, 8, ETOK])

    wc = din("wc", [E, DIM])
    bcT = din("bcT", [128, 8], F32)
    lng = din("lng", [128, 19, 8], F32)
    lnb = din("lnb", [128, 19, 8], F32)
    wqkv = din("wqkv", [L, DIM + 1, 3 * DIM])     # row DIM = bqkv (for V bias)
    bqkT = din("bqkT", [128, L, 16], F32)
    wo = din("wo", [L, DIM + 1, DIM])             # row DIM = bo
    co = din("co", [L, DIM + 1, DIM])             # row DIM = cbo
    cq = din("cq", [L, DIM, DIM])
    cbqT = din("cbqT", [128, L, 8], F32)
    ckv = din("ckv", [L, DIM + 1, 2 * DIM])       # row DIM = cbkv (for V bias)
    cbkvkT = din("cbkvkT", [128, L, 8], F32)
    w1 = din("w1", [L, DIM, FF])
    b1T = din("b1T", [128, L, 16], F32)
    w2 = din("w2", [L, FF + 1, DIM])              # row FF = b2
    wnar = din("wnar", [DIM, SHRINK * HALF])
    bnarT = din("bnarT", [128, 20], F32)

    out_cs = nc.dram_tensor("out_cs", [128, 20 * TOK], BF16, kind="ExternalOutput")

    with tile.TileContext(nc) as tc:
      with (
        tc.tile_pool(name="const", bufs=1) as cp,
        tc.tile_pool(name="xs", bufs=1) as xs,
        tc.tile_pool(name="lnw", bufs=1) as lnw,
        tc.tile_pool(name="psA", bufs=1, space="PSUM") as psA,
      ):
        ones = cp.tile([1, 512], BF16)
        onesF = cp.tile([1, 128], F32)
        onesD = cp.tile([128, 1], BF16)
        epst = cp.tile([1, 1], F32)
        lngt = cp.tile([128, 19, 8], F32)
        lnbt = cp.tile([128, 19, 8], F32)
        nc.vector.memset(ones[:], 1.0)
        nc.vector.memset(onesF[:], 1.0)
        nc.vector.memset(onesD[:], 1.0 / DIM)
        nc.vector.memset(epst[:], 1e-12)
        nc.sync.dma_start(lngt[:], lng[:])
        nc.sync.dma_start(lnbt[:], lnb[:])

        x = xs.tile([128, 8, TOK], BF16)
        s_t = xs.tile([128, 8, TOK], BF16)

        def pst(tag):
            return psA.tile([128, 512], F32, tag=tag, name="ps_" + tag)

        def layernorm(xout, sin, ln_idx, sq_t):
            nc.scalar.activation(sq_t[:], sin[:], AF.Square)
            for w0 in (0, 400):
                wn = 400
                mps = pst("t4")[:1, :wn]
                qps = pst("t5")[:1, :wn]
                for kc in range(8):
                    nc.tensor.matmul(mps, onesD[:], sin[:, kc, w0:w0 + wn],
                                     start=(kc == 0), stop=(kc == 7))
                for kc in range(8):
                    nc.tensor.matmul(qps, onesD[:], sq_t[:, kc, w0:w0 + wn],
                                     start=(kc == 0), stop=(kc == 7))
                mean_s = lnw.tile([1, 400], F32, tag="ln_mean")
                m2 = lnw.tile([1, 400], F32, tag="ln_m2")
                istd = lnw.tile([1, 400], F32, tag="ln_istd")
                mi = lnw.tile([1, 400], F32, tag="ln_mi")
                nc.vector.tensor_copy(mean_s[:], mps)
                nc.vector.tensor_tensor(m2[:], mean_s[:], mean_s[:], ALU.mult)
                nc.vector.tensor_tensor(m2[:], qps, m2[:], ALU.subtract)
                nc.scalar.activation(m2[:], m2[:], AF.Sqrt, bias=epst[:])
                nc.vector.reciprocal(istd[:], m2[:])
                nc.vector.tensor_tensor(mi[:], mean_s[:], istd[:], ALU.mult)
                ibc = pst("t6")[:, :wn]
                mbc = pst("t7")[:, :wn]
                nc.tensor.matmul(ibc, onesF[:, 0:128], istd[:], start=True, stop=True)
                nc.tensor.matmul(mbc, onesF[:, 0:128], mi[:], start=True, stop=True)
                for kc in range(8):
                    t1 = lnw.tile([128, 400], BF16, tag="ln_t1")
                    nc.vector.tensor_tensor(t1[:], sin[:, kc, w0:w0 + wn], ibc, ALU.mult)
                    nc.vector.tensor_tensor(t1[:], t1[:], mbc, ALU.subtract)
                    nc.vector.tensor_scalar(
                        xout[:, kc, w0:w0 + wn], t1[:],
                        lngt[:, ln_idx, kc:kc + 1], lnbt[:, ln_idx, kc:kc + 1],
                        ALU.mult, ALU.add)

        # ================= phase 0: conv + pool + embed =================
        with (
            tc.tile_pool(name="p0", bufs=1) as p0,
            tc.tile_pool(name="p0w", bufs=1) as p0w,
        ):
            embt = p0.tile([E, CTOK], BF16)
            nc.sync.dma_start(embt[:], embT[:])
            wcs = p0w.tile([E, DIM], BF16, tag="wc")
            nc.sync.dma_start(wcs[:], wc[:])
            bct = p0w.tile([128, 8], F32, tag="bct")
            nc.sync.dma_start(bct[:], bcT[:])
            pos = p0.tile([128, 8, TOK], BF16)
            nc.sync.dma_start(pos[:], posT[:])
            cmsk = p0w.tile([1, CTOK], F32, tag="cmsk")
            nc.sync.dma_start(cmsk[:], convmask[:])
            mbc_t = p0.tile([128, CTOK], BF16)
            for wi, (w0, wn) in enumerate(_tokwins(CTOK, 500)):
                mb = pst("t2")[:, :wn]
                nc.tensor.matmul(mb, onesF[:, 0:128], cmsk[:, w0:w0 + wn],
                                 start=True, stop=True)
                nc.scalar.copy(mbc_t[:, w0:w0 + wn], mb)

            for mc in range(8):
                hm = p0.tile([128, CTOK], BF16, tag="hconv")
                for wi, (w0, wn) in enumerate(_tokwins(CTOK, 500)):
                    ps = pst("t0" if wi % 2 == 0 else "t1")[:, :wn]
                    nc.tensor.matmul(ps, wcs[:, mc * 128:(mc + 1) * 128],
                                     embt[:, w0:w0 + wn], start=True, stop=True)
                    nc.scalar.activation(hm[:, w0:w0 + wn], ps, AF.Relu,
                                         bias=bct[:, mc:mc + 1])
                nc.vector.tensor_tensor(hm[:], hm[:], mbc_t[:], ALU.mult)
                nc.vector.tensor_reduce(
                    s_t[:, mc, :], hm[:].rearrange("p (t s) -> p t s", s=SHRINK),
                    mybir.AxisListType.X, ALU.max)
            nc.vector.tensor_tensor(s_t[:], s_t[:], pos[:], ALU.add)
            sq0 = p0.tile([128, 8, TOK], BF16)
            layernorm(x, s_t, 0, sq0)

        # ================= phases 1+2: bert layers + nar =================
        with (
            tc.tile_pool(name="wsl", bufs=18) as wsl,
            tc.tile_pool(name="wbias", bufs=2) as wbias,
            tc.tile_pool(name="vsl", bufs=1) as vsl,
            tc.tile_pool(name="qkv", bufs=1) as qkvp,
            tc.tile_pool(name="att", bufs=3) as attp,
            tc.tile_pool(name="f1p", bufs=1) as f1p,
            tc.tile_pool(name="bias", bufs=1) as biasp,
        ):
            QT = qkvp.tile([128, 8, TOK], BF16)
            KT = qkvp.tile([128, 8, TOK], BF16)
            Vt = qkvp.tile([128, BC, 2, HEADS, 65], BF16)
            KcT = qkvp.tile([128, 8, ETOK], BF16)
            Vct = qkvp.tile([128, BC, 2, HEADS, 65], BF16)
            attnT = qkvp.tile([128, 8, TOK], BF16)
            f1T = f1p.tile([128, 16, TOK], BF16)
            enct = qkvp.tile([128, 8, ETOK], BF16)
            smt = qkvp.tile([128, 2, BC, T], BF16)
            cmt = qkvp.tile([128, 2, BC], F32)
            bqk_t = biasp.tile([128, L, 16], F32)
            cbq_t = biasp.tile([128, L, 8], F32)
            cbkvk_t = biasp.tile([128, L, 8], F32)
            b1_t = biasp.tile([128, L, 16], F32)
            bnar_t = biasp.tile([128, 20], F32)
            nc.sync.dma_start(smt[:], selfmaskT[:])
            nc.sync.dma_start(cmt[:], crossmaskT[:])
            nc.sync.dma_start(enct[:], encT[:])
            nc.sync.dma_start(bqk_t[:], bqkT[:])
            nc.sync.dma_start(cbq_t[:], cbqT[:])
            nc.sync.dma_start(cbkvk_t[:], cbkvkT[:])
            nc.sync.dma_start(b1_t[:], b1T[:])
            nc.sync.dma_start(bnar_t[:], bnarT[:])

            def gemm_b(wdram, kcs, rhs_fn, mcs, out_cb, bias_row=None,
                       ntok=TOK, ntw=512):
                """feature-major GEMM: psum[mc] = sum_kc W[kc,:,mc*128:]^T @ rhs(kc)"""
                M = wdram.shape[-1]
                brow = None
                if bias_row is not None:
                    brow = wbias.tile([1, M], BF16, tag="wbias", name="brow")
                    nc.sync.dma_start(brow[:], wdram[bias_row:bias_row + 1, :])
                for mc in range(mcs):
                    wts = []
                    for kc in range(kcs):
                        wt = wsl.tile([128, 128], BF16, tag="wt", name="wt")
                        nc.sync.dma_start(
                            wt[:], wdram[kc * 128:(kc + 1) * 128,
                                         mc * 128:(mc + 1) * 128])
                        wts.append(wt)
                    for wi, (w0, wn) in enumerate(_tokwins(ntok, ntw)):
                        p = pst("t0" if (mc + wi) % 2 == 0 else "t1")[:, :wn]
                        for kc in range(kcs):
                            nc.tensor.matmul(
                                p, wts[kc], rhs_fn(kc, w0, wn),
                                start=(kc == 0),
                                stop=(kc == kcs - 1 and brow is None))
                        if brow is not None:
                            nc.tensor.matmul(p, brow[:, mc * 128:(mc + 1) * 128],
                                             ones[0:1, :wn], start=False, stop=True)
                        out_cb(mc, w0, wn, p)

            def gemm_a_v(wdram_v, rhs_x, vdst, ntok_grp, bias_row):
                """token-major V gemm: for each batch b and kp-chunk, psum
                [kn, 512] = x_chunk^T @ Wv, written into vdst[., b, chunk, h, d]."""
                slabs = []
                for kc in range(8):
                    sl = vsl.tile([128, DIM], BF16, tag=f"wslV{kc}", name="sl")
                    nc.sync.dma_start(sl[:], wdram_v[kc * 128:(kc + 1) * 128, :])
                    slabs.append(sl)
                brow = vsl.tile([1, DIM], BF16, tag="wslVb", name="brow")
                nc.sync.dma_start(brow[:], wdram_v[bias_row:bias_row + 1, :])
                kplens = (128, ntok_grp - 128)
                for b in range(BC):
                    for kchunk in range(2):
                        kn = kplens[kchunk]
                        t0 = b * ntok_grp + kchunk * 128
                        for w in range(2):
                            p = pst("t0" if w == 0 else "t1")[:kn, :]
                            for kc in range(8):
                                nc.tensor.matmul(
                                    p, rhs_x(kc, t0, kn),
                                    slabs[kc][:, w * 512:(w + 1) * 512],
                                    start=(kc == 0), stop=False)
                            nc.tensor.matmul(p, ones[0:1, :kn],
                                             brow[:, w * 512:(w + 1) * 512],
                                             start=False, stop=True)
                            nc.scalar.copy(
                                vdst[:kn, b, kchunk, w * 8:(w + 1) * 8, 0:64],
                                p.rearrange("k (h d) -> k h d", d=64))
                # ones column for the fused sum-of-exp row
                nc.vector.memset(vdst[:, :, :, :, 64:65], 1.0)

            def attention(src_is_self):
                kt = KT if src_is_self else KcT
                vt = Vt if src_is_self else Vct
                nkp = T if src_is_self else SRC
                kplens = (128, nkp - 128)
                for b in range(BC):
                    for h in range(HEADS):
                        hp, hc = (h % 2) * 64, h // 2
                        et = attp.tile([128, 2, T], BF16, tag="et")
                        pso = pst("t2")[:65, :T]
                        for kchunk in range(2):
                            kn = kplens[kchunk]
                            base = b * nkp + kchunk * 128
                            pss = pst("t0" if kchunk == 0 else "t1")[:kn, :T]
                            nc.tensor.matmul(
                                pss, kt[hp:hp + 64, hc, base:base + kn],
                                QT[hp:hp + 64, hc, b * T:(b + 1) * T],
                                start=True, stop=True)
                            nc.scalar.activation(et[:kn, kchunk, :], pss, AF.Exp,
                                                 scale=SCALE)
                            if src_is_self:
                                nc.vector.tensor_tensor(
                                    et[:kn, kchunk, :], et[:kn, kchunk, :],
                                    smt[:kn, kchunk, b, :], ALU.mult)
                            else:
                                nc.vector.tensor_scalar_mul(
                                    et[:kn, kchunk, :], et[:kn, kchunk, :],
                                    cmt[:kn, kchunk, b:b + 1])
                            nc.tensor.matmul(
                                pso, vt[:kn, b, kchunk, h, :], et[:kn, kchunk, :],
                                start=(kchunk == 0), stop=(kchunk == 1))
                        rc = attp.tile([1, T], F32, tag="rc")
                        nc.vector.reciprocal(rc[:], pso[64:65, :])
                        prb = pst("t3")[:64, :T]
                        nc.tensor.matmul(prb, onesF[:, 0:64], rc[:],
                                         start=True, stop=True)
                        rb = attp.tile([64, T], F32, tag="rb")
                        nc.scalar.copy(rb[:], prb)
                        nc.vector.tensor_tensor(
                            attnT[hp:hp + 64, hc, b * T:(b + 1) * T],
                            pso[0:64, :], rb[:], ALU.mult)

            x_rhs = lambda kc, w0, wn: x[:, kc, w0:w0 + wn]

            for l in range(L):
                # ---- self attention ----
                def qk_out(mc, w0, wn, p, l=l):
                    dst = QT if mc < 8 else KT
                    nc.scalar.activation(dst[:, mc % 8, w0:w0 + wn], p, AF.Identity,
                                         bias=bqk_t[:, l, mc:mc + 1])
                gemm_b(wqkv[l, :, 0:2 * DIM], 8, x_rhs, 16, qk_out)
                gemm_a_v(wqkv[l, :, 2 * DIM:3 * DIM],
                         lambda kc, t0, kn: x[:, kc, t0:t0 + kn], Vt, T, DIM)
                attention(True)

                def o_out(mc, w0, wn, p):
                    nc.vector.tensor_tensor(s_t[:, mc, w0:w0 + wn], p,
                                            x[:, mc, w0:w0 + wn], ALU.add)
                gemm_b(wo[l], 8,
                       lambda kc, w0, wn: attnT[:, kc, w0:w0 + wn], 8, o_out,
                       bias_row=DIM)
                layernorm(x, s_t, 1 + 3 * l, attnT)

                # ---- cross attention ----
                def q_out(mc, w0, wn, p, l=l):
                    nc.scalar.activation(QT[:, mc, w0:w0 + wn], p, AF.Identity,
                                         bias=cbq_t[:, l, mc:mc + 1])
                gemm_b(cq[l], 8, x_rhs, 8, q_out)

                def kc_out(mc, w0, wn, p, l=l):
                    nc.scalar.activation(KcT[:, mc, w0:w0 + wn], p, AF.Identity,
                                         bias=cbkvk_t[:, l, mc:mc + 1])
                gemm_b(ckv[l][:, 0:DIM], 8,
                       lambda kc, w0, wn: enct[:, kc, w0:w0 + wn], 8, kc_out,
                       ntok=ETOK)
                gemm_a_v(ckv[l][:, DIM:2 * DIM],
                         lambda kc, t0, kn: enct[:, kc, t0:t0 + kn], Vct, SRC, DIM)
                attention(False)

                def co_out(mc, w0, wn, p):
                    nc.vector.tensor_tensor(s_t[:, mc, w0:w0 + wn], p,
                                            x[:, mc, w0:w0 + wn], ALU.add)
                gemm_b(co[l], 8,
                       lambda kc, w0, wn: attnT[:, kc, w0:w0 + wn], 8, co_out,
                       bias_row=DIM)
                layernorm(x, s_t, 2 + 3 * l, attnT)

                # ---- ffn ----
                def f1_out(mc, w0, wn, p, l=l):
                    nc.scalar.activation(f1T[:, mc, w0:w0 + wn], p, AF.Relu,
                                         bias=b1_t[:, l, mc:mc + 1])
                gemm_b(w1[l], 8, x_rhs, 16, f1_out)

                def f2_out(mc, w0, wn, p):
                    nc.vector.tensor_tensor(s_t[:, mc, w0:w0 + wn], p,
                                            x[:, mc, w0:w0 + wn], ALU.add)
                gemm_b(w2[l], 16,
                       lambda kc, w0, wn: f1T[:, kc, w0:w0 + wn], 8, f2_out,
                       bias_row=FF)
                layernorm(x, s_t, 3 + 3 * l, attnT)

            # ---- nar projection -> char states (to DRAM for host tail) ----
            ocs = out_cs[:].rearrange("p (c t) -> p c t", t=TOK)

            def nar_out(mc, w0, wn, p):
                stg = f1p.tile([128, 512], BF16, tag="narstg")
                nc.scalar.activation(stg[:, :wn], p, AF.Identity,
                                     bias=bnar_t[:, mc:mc + 1])
                nc.sync.dma_start(ocs[:, mc, w0:w0 + wn], stg[:, :wn])
            gemm_b(wnar, 8, x_rhs, 20, nar_out)

    nc.compile()
    return nc
